# revision 7
# baseline (speedup 1.0000x reference)
"""Trainium2 Bass kernel for nn_AttentionHead (B=2, S=2048, D=768, H=12).

Sharding: 8 cores = 2 batches x 4 head-groups (3 heads each).
Per core: QKV projection for its heads (transposed layout), causal
attention with softmax over the QUERY axis (reference peculiarity:
softmax dim=-2, scaled by sqrt(d_model)), AllGather of per-head outputs
within each batch's 4-core group, then a column-slice of the output
projection.  Host only slices / transposes / concatenates.

v2 structure (vs v1):
  - DMA priority order (wqk, then x in 512-col chunks, wv, rest) with
    PE warm-up matmuls covering the load phase and QKV chunks starting
    as soon as the first x columns land -- keeps the PE HAM clock warm.
  - waveA (heads 0,1) runs FIRST so its per-512-block AllGathers fire
    early; waveB (head 2) runs second with the output projection
    interleaved per block as the B gathers complete.
  - Software pipelining: AV(ki-1) is emitted between the score halves
    of ki so the PE has work while ScalarE exps; v_tiles/qkv2 chunks
    fill remaining PE slack.
  - The causal diagonal mask is applied on the PE (identity @ triT
    accumulated into the score PSUM) instead of a DVE add.
  - waveB AV packs the two 1024-column halves into PE column groups
    (partitions 0-63 / 64-127), halving its AV wall time and shrinking
    the accumulator to 2 PSUM banks.
"""

import math

import numpy as np

B, S, D, H, DH = 2, 2048, 768, 12, 64
NCORES = 8
GROUPS = 4  # head-groups per batch
HPG = 3  # heads per group
EPG = HPG * DH  # 192
SCALE = 1.0 / math.sqrt(D)
NEG = -1.0e30

_cache = {}


def _build(causal: bool):
    import contextlib

    import concourse.bacc as bacc
    import concourse.mybir as mybir
    from concourse import tile

    f32 = mybir.dt.float32
    bf16 = mybir.dt.bfloat16
    EXP = mybir.ActivationFunctionType.Exp

    nc = bacc.Bacc("TRN2", target_bir_lowering=False, debug=False, num_devices=NCORES)

    xT = nc.dram_tensor("xT", [D, S], f32, kind="ExternalInput")
    wqk = nc.dram_tensor("wqk", [D, 384], f32, kind="ExternalInput")
    wv = nc.dram_tensor("wv", [D, EPG], f32, kind="ExternalInput")
    bqkc = nc.dram_tensor("bqkc", [384, 1], f32, kind="ExternalInput")
    bv = nc.dram_tensor("bv", [1, EPG], f32, kind="ExternalInput")
    wout = nc.dram_tensor("wout", [D, EPG], f32, kind="ExternalInput")
    boutc = nc.dram_tensor("boutc", [EPG, 1], f32, kind="ExternalInput")
    tri = nc.dram_tensor("tri", [128, 128], f32, kind="ExternalInput")
    ident = nc.dram_tensor("ident", [128, 128], f32, kind="ExternalInput")
    out = nc.dram_tensor("out", [EPG, S], f32, kind="ExternalOutput")

    ag_inA = [nc.dram_tensor(f"ag_inA{f}", [128, 512], bf16) for f in range(4)]
    ag_outA = [nc.dram_tensor(f"ag_outA{f}", [512, 512], bf16) for f in range(4)]
    ag_inB = [nc.dram_tensor(f"ag_inB{f}", [64, 512], bf16) for f in range(4)]
    ag_outB = [nc.dram_tensor(f"ag_outB{f}", [256, 512], bf16) for f in range(4)]

    groups = [[0, 1, 2, 3], [4, 5, 6, 7]]

    with tile.TileContext(nc) as tc:
        with contextlib.ExitStack() as ctx:
            const_p = ctx.enter_context(tc.tile_pool(name="const", bufs=1))
            w_p = ctx.enter_context(tc.tile_pool(name="w", bufs=6))
            xs_p = ctx.enter_context(tc.tile_pool(name="xs", bufs=6))
            xt_p = ctx.enter_context(tc.tile_pool(name="xt", bufs=1))
            qk_p = ctx.enter_context(tc.tile_pool(name="qk", bufs=1))
            v_p = ctx.enter_context(tc.tile_pool(name="v", bufs=1))
            e_p = ctx.enter_context(tc.tile_pool(name="e", bufs=10))
            st_p = ctx.enter_context(tc.tile_pool(name="stat", bufs=24))
            vp_p = ctx.enter_context(tc.tile_pool(name="vp", bufs=6))
            atn_p = ctx.enter_context(tc.tile_pool(name="atn", bufs=1))
            ag_p = ctx.enter_context(tc.tile_pool(name="ag", bufs=1))
            o_p = ctx.enter_context(tc.tile_pool(name="oT", bufs=2))
            psS = ctx.enter_context(tc.tile_pool(name="psS", bufs=2, space="PSUM"))

            # ---- constants ----
            ones_f = const_p.tile([1, 512], f32)
            nc.vector.memset(ones_f[:], 1.0)
            ones = const_p.tile([1, 512], bf16)
            nc.vector.tensor_copy(ones[:], ones_f[:])
            tri_f = const_p.tile([128, 128], f32)
            nc.sync.dma_start(tri_f[:], tri[:, :])
            tri_b = const_p.tile([128, 128], bf16)
            nc.vector.tensor_copy(tri_b[:], tri_f[:])
            id_f = const_p.tile([128, 128], f32)
            nc.sync.dma_start(id_f[:], ident[:, :])
            id_b = const_p.tile([128, 128], bf16)
            nc.vector.tensor_copy(id_b[:], id_f[:])

            bqk_c = const_p.tile([128, 3], f32)
            nc.sync.dma_start(bqk_c[:], bqkc[:, :].rearrange("(c p) o -> p (c o)", p=128))
            bout_c = const_p.tile([128, 2], f32)
            nc.sync.dma_start(bout_c[0:64, 1:2], boutc[128:EPG, :])
            nc.sync.dma_start(bout_c[:, 0:1], boutc[0:128, :])
            bv_f = const_p.tile([1, EPG], f32)
            nc.sync.dma_start(bv_f[:], bv[:, :])
            bv_t = const_p.tile([1, EPG], bf16)
            nc.vector.tensor_copy(bv_t[:], bv_f[:])

            # ---- PE warm-up: dummy matmuls while the first DMAs land ----
            warm_ctx = tc.tile_pool(name="psW", bufs=1, space="PSUM")
            psW = warm_ctx.__enter__()
            warm_in = const_p.tile([128, 512], bf16)
            nc.vector.memset(warm_in[:], 0.0)
            for wi in range(22):
                wps = psW.tile([128, 512], f32, tag="warm")
                nc.tensor.matmul(
                    wps[:], id_b[:], warm_in[:],
                    start=True, stop=True, skip_group_check=True,
                )

            # ---- staged weight/x DMAs in priority order + casts ----
            # wqk first (needed by the first qkv chunks), then x column
            # chunks, wv, then wout last.
            wqk_t, wv_t, wout_t = [], [], []
            for dt_i in range(6):
                wf = w_p.tile([128, 384], f32, tag="wstg")
                nc.sync.dma_start(wf[:], wqk[dt_i * 128 : (dt_i + 1) * 128, :])
                wt = w_p.tile([128, 384], bf16, tag="wqk")
                nc.vector.tensor_copy(wt[:], wf[:])
                wqk_t.append(wt)

            xt_t = [
                xt_p.tile([128, S], bf16, tag=f"xt{i}", name=f"xt{i}")
                for i in range(6)
            ]

            def x_chunk(dt_i, sc, on_scalar=False):
                xs = xs_p.tile([128, 512], f32, tag="xstg")
                nc.sync.dma_start(
                    xs[:], xT[dt_i * 128 : (dt_i + 1) * 128, sc * 512 : (sc + 1) * 512]
                )
                dst = xt_t[dt_i][:, sc * 512 : (sc + 1) * 512]
                if on_scalar:
                    nc.scalar.copy(dst, xs[:])
                else:
                    nc.vector.tensor_copy(dst, xs[:])

            # qkv chunk c (128 rows of [k0k1|q0q1|q2k2]) for seq cols
            # [sc*512, (sc+1)*512); bias folded into the PSUM evacuation.
            def qkv_chunk(dst, c, sc):
                pt = psS.tile([128, 1024], f32, tag="strip")
                for dt_i in range(6):
                    nc.tensor.matmul(
                        pt[:, 0:512],
                        wqk_t[dt_i][:, c * 128 : (c + 1) * 128],
                        xt_t[dt_i][:, sc * 512 : (sc + 1) * 512],
                        start=(dt_i == 0), stop=(dt_i == 5),
                    )
                nc.vector.tensor_scalar_add(
                    dst[:, sc * 512 : (sc + 1) * 512], pt[:, 0:512], bqk_c[:, c : c + 1]
                )

            vnat = v_p.tile([128, 16 * EPG], bf16)

            def v_tile(st_i):
                ptf = psS.tile([128, 1024], f32, tag="strip")
                p = ptf[:, 0:EPG]
                nc.tensor.matmul(p, ones[:, 0:128], bv_t[:], start=True, stop=False)
                for dt_i in range(6):
                    nc.tensor.matmul(
                        p,
                        xt_t[dt_i][:, st_i * 128 : (st_i + 1) * 128],
                        wv_t[dt_i][:],
                        start=False, stop=(dt_i == 5),
                    )
                nc.vector.tensor_copy(vnat[:, st_i * EPG : (st_i + 1) * EPG], p)

            k01 = qk_p.tile([128, S], bf16, tag="k01")
            q01 = qk_p.tile([128, S], bf16, tag="q01")
            qk2 = qk_p.tile([128, S], bf16, tag="qk2")
            k2 = qk_p.tile([64, S], bf16, tag="k2")
            q2d = qk_p.tile([128, S], bf16, tag="q2d")

            # phase 1: x chunks + qkv01 + first v_tiles, priority-ordered
            for dt_i in range(6):
                x_chunk(dt_i, 0)
            qkv_chunk(k01, 0, 0)
            qkv_chunk(q01, 1, 0)
            for dt_i in range(6):
                x_chunk(dt_i, 1, on_scalar=True)
            # wv (for v_tiles) after x sc0/sc1
            for dt_i in range(6):
                vf = w_p.tile([128, EPG], f32, tag="wvstg")
                nc.sync.dma_start(vf[:], wv[dt_i * 128 : (dt_i + 1) * 128, :])
                vt = w_p.tile([128, EPG], bf16, tag="wv")
                nc.gpsimd.tensor_copy(vt[:], vf[:])
                wv_t.append(vt)
            qkv_chunk(k01, 0, 1)
            qkv_chunk(q01, 1, 1)
            for dt_i in range(6):
                x_chunk(dt_i, 2)
            v_tile(0)
            v_tile(1)
            qkv_chunk(k01, 0, 2)
            qkv_chunk(q01, 1, 2)
            for dt_i in range(6):
                x_chunk(dt_i, 3, on_scalar=True)
            # wout last (needed only at oproj time)
            for dt_i in range(6):
                wos = w_p.tile([128, EPG], f32, tag="wostg")
                nc.sync.dma_start(wos[:], wout[dt_i * 128 : (dt_i + 1) * 128, :])
                wo = w_p.tile([128, EPG], bf16, tag="wout")
                nc.gpsimd.tensor_copy(wo[:], wos[:])
                wout_t.append(wo)
            v_tile(2)
            qkv_chunk(k01, 0, 3)
            qkv_chunk(q01, 1, 3)
            warm_ctx.__exit__(None, None, None)

            def halves_of(ki):
                q0 = 128 * ki if causal else 0
                L = S - q0
                hs = [(q0, min(L, 1024))]
                if L > 1024:
                    hs.append((q0 + 1024, L - 1024))
                return hs

            def make_vpt(accs, ki, head):
                rcp = st_p.tile([128, 1], f32, tag="rcp")
                if len(accs) == 2:
                    ssum = st_p.tile([128, 1], f32, tag="ssum")
                    nc.vector.tensor_add(ssum[:], accs[0][:], accs[1][:])
                    nc.vector.reciprocal(rcp[:], ssum[:])
                else:
                    nc.vector.reciprocal(rcp[:], accs[0][:])
                vpt = vp_p.tile([128, 64], bf16, tag="vp")
                nc.vector.tensor_scalar_mul(
                    vpt[:],
                    vnat[:, ki * EPG + head * 64 : ki * EPG + (head + 1) * 64],
                    rcp[:],
                )
                return vpt

            agA_t = [
                ag_p.tile([128, S], bf16, tag=f"agA{i}", name=f"agA{i}")
                for i in range(4)
            ]
            agB_t = [
                ag_p.tile([128, S], bf16, tag=f"agB{i}", name=f"agB{i}")
                for i in range(2)
            ]

            atnA = atn_p.tile([128, S], bf16, tag="atnA")
            atn2 = atn_p.tile([64, S], bf16, tag="atn2")

            def flushA(f, avA):
                cols = slice(512 * f, 512 * (f + 1))
                nc.vector.tensor_copy(atnA[:, cols], avA[:, cols])
                nc.sync.dma_start(ag_inA[f][:, :], atnA[:, cols])
                nc.gpsimd.collective_compute(
                    "AllGather",
                    mybir.AluOpType.bypass,
                    replica_groups=groups,
                    ins=[ag_inA[f].ap().opt()],
                    outs=[ag_outA[f].ap().opt()],
                )
                for dt_i in range(4):
                    nc.sync.dma_start(
                        agA_t[dt_i][:, cols],
                        ag_outA[f][dt_i * 128 : (dt_i + 1) * 128, :],
                    )

            def flushB(f, av2):
                cols = slice(512 * f, 512 * (f + 1))
                if causal:
                    src = av2[0:64, cols] if f < 2 else av2[64:128, slice(512 * (f - 2), 512 * (f - 1))]
                else:
                    src = av2[0:64, cols] if f < 2 else av2[64:128, slice(512 * (f - 2), 512 * (f - 1))]
                nc.vector.tensor_copy(atn2[:, cols], src)
                nc.sync.dma_start(ag_inB[f][:, :], atn2[:, cols])
                nc.gpsimd.collective_compute(
                    "AllGather",
                    mybir.AluOpType.bypass,
                    replica_groups=groups,
                    ins=[ag_inB[f].ap().opt()],
                    outs=[ag_outB[f].ap().opt()],
                )
                for dt_i in range(2):
                    nc.sync.dma_start(
                        agB_t[dt_i][:, cols],
                        ag_outB[f][dt_i * 128 : (dt_i + 1) * 128, :],
                    )

            # ---- wave A: heads 0+1, row/col-group paired, pipelined ----
            psA_ctx = tc.tile_pool(name="psA", bufs=1, space="PSUM")
            psA = psA_ctx.__enter__()
            avA = psA.tile([128, S], f32, tag="avA")

            def emit_avA(ki, ets, vpts):
                for hv, (h0, hl, et0, et1) in enumerate(ets):
                    off = 0
                    while off < hl:
                        n = min(512, hl - off)
                        for hi, et in ((0, et0), (1, et1)):
                            nc.tensor.matmul(
                                avA[64 * hi : 64 * hi + 64, h0 + off : h0 + off + n],
                                vpts[hi][:],
                                et[:, off : off + n],
                                start=(ki == 0),
                                stop=(ki == 15),
                                skip_group_check=True,
                            )
                        off += n

            prev = None  # (ki, ets, vpts)
            for ki in range(16):
                hs = halves_of(ki)
                cur_ets = []
                accs = {0: [], 1: []}
                for hv, (h0, hl) in enumerate(hs):
                    s0 = psS.tile([128, 1024], f32, tag="strip")
                    s1 = psS.tile([128, 1024], f32, tag="strip")
                    off = 0
                    while off < hl:
                        n = min(512, hl - off)
                        diag = causal and hv == 0 and off == 0
                        for hi, s in ((0, s0), (1, s1)):
                            nc.tensor.matmul(
                                s[:, off : off + n],
                                k01[64 * hi : 64 * hi + 64, ki * 128 : (ki + 1) * 128],
                                q01[64 * hi : 64 * hi + 64, h0 + off : h0 + off + n],
                                start=True,
                                stop=not diag,
                                skip_group_check=True,
                            )
                        if diag:
                            for s in (s0, s1):
                                nc.tensor.matmul(
                                    s[:, 0:128], id_b[:], tri_b[:],
                                    start=False, stop=True, skip_group_check=True,
                                )
                        off += n
                    if hv == 0:
                        # PE filler while ScalarE exps this ki: AV of ki-1,
                        # then one interleave job.
                        if prev is not None:
                            emit_avA(prev[0], prev[1], prev[2])
                        if ki + 3 <= 15:
                            v_tile(ki + 3)
                        if ki >= 8 and ki % 2 == 0:
                            sc = (ki - 8) // 2
                            qkv_chunk(qk2, 2, sc)
                            cols = slice(sc * 512, (sc + 1) * 512)
                            nc.gpsimd.dma_start(k2[:, cols], qk2[64:128, cols])
                            nc.gpsimd.dma_start(q2d[64:128, cols], qk2[0:64, cols])
                    ets_half = []
                    for hi, s in ((0, s0), (1, s1)):
                        et = e_p.tile([128, 1024], bf16, tag="e")
                        acc = st_p.tile([128, 1], f32, tag="acc")
                        nc.scalar.activation(
                            et[:, 0:hl], s[:, 0:hl], EXP,
                            scale=SCALE, accum_out=acc[:],
                        )
                        ets_half.append(et)
                        accs[hi].append(acc)
                    cur_ets.append((h0, hl, ets_half[0], ets_half[1]))
                vpts = [make_vpt(accs[hi], ki, hi) for hi in range(2)]
                # flush completed 512-col blocks (block f done after AV(4f+3),
                # which was emitted during ki=4f+4's hv0 above)
                if causal and ki >= 4 and (ki % 4) == 0:
                    flushA(ki // 4 - 1, avA)
                prev = (ki, cur_ets, vpts)
            emit_avA(prev[0], prev[1], prev[2])
            if causal:
                flushA(3, avA)
            else:
                for f in range(4):
                    flushA(f, avA)
            psA_ctx.__exit__(None, None, None)

            # ---- wave B: head 2, ki pairs in row groups, col-half-packed AV
            psB_ctx = tc.tile_pool(name="psB", bufs=1, space="PSUM")
            psB = psB_ctx.__enter__()
            psO_ctx = tc.tile_pool(name="psO", bufs=2, space="PSUM")
            psO = psO_ctx.__enter__()
            av2 = psB.tile([128, 1024], f32, tag="av2")

            def emit_av2(ki, ets, vpt):
                # col-half packing: abs cols [0,1024) -> av2[0:64],
                # [1024,2048) -> av2[64:128, col-1024]; chunks may not
                # cross the 1024 boundary.
                for (h0, hl, et) in ets:
                    off = 0
                    while off < hl:
                        col = h0 + off
                        n = min(512, hl - off)
                        if col < 1024:
                            n = min(n, 1024 - col)
                            dst = av2[0:64, col : col + n]
                        else:
                            dst = av2[64:128, col - 1024 : col - 1024 + n]
                        nc.tensor.matmul(
                            dst,
                            vpt[:],
                            et[:, off : off + n],
                            start=(ki == 0),
                            stop=(ki == 15),
                            skip_group_check=True,
                        )
                        off += n

            def oproj(f):
                cols = slice(512 * f, 512 * (f + 1))
                src_t = agA_t + agB_t
                for mc, (m0, mw) in enumerate([(0, 128), (128, 64)]):
                    pt = psO.tile([128, 512], f32, tag="po")
                    for dt_i in range(6):
                        nc.tensor.matmul(
                            pt[0:mw, :],
                            wout_t[dt_i][:, m0 : m0 + mw],
                            src_t[dt_i][:, cols],
                            start=(dt_i == 0), stop=(dt_i == 5),
                        )
                    oT = o_p.tile([128, 512], f32, tag="oT")
                    nc.vector.tensor_scalar_add(
                        oT[0:mw, :], pt[0:mw, :], bout_c[0:mw, mc : mc + 1]
                    )
                    nc.sync.dma_start(out[m0 : m0 + mw, cols], oT[0:mw, :])

            srcs = {}
            for ki in range(16):
                if ki % 2 == 0:
                    srcs[ki] = (k2, 0, qk2, 0)
                else:
                    srcs[ki] = (qk2, 64, q2d, 64)

            prevB = None  # (kis, {ki: (ets, vpt)})
            for t in range(8):
                kis = (2 * t, 2 * t + 1)
                ets = {ki: [] for ki in kis}
                accs = {ki: [] for ki in kis}
                maxhv = max(len(halves_of(ki)) for ki in kis)
                for hv in range(maxhv):
                    batch = []
                    for ki in kis:
                        hsk = halves_of(ki)
                        if hv < len(hsk):
                            batch.append((ki, hsk[hv]))
                    s_list = [
                        psS.tile([128, 1024], f32, tag="strip", name="sB")
                        for _ in batch
                    ]
                    maxhl = max(hl for _, (_, hl) in batch)
                    off = 0
                    while off < maxhl:
                        for s, (ki, (h0, hl)) in zip(s_list, batch):
                            if off >= hl:
                                continue
                            n = min(512, hl - off)
                            kT, kb, qT, qb = srcs[ki]
                            diag = causal and hv == 0 and off == 0
                            nc.tensor.matmul(
                                s[:, off : off + n],
                                kT[kb : kb + 64, ki * 128 : (ki + 1) * 128],
                                qT[qb : qb + 64, h0 + off : h0 + off + n],
                                start=True,
                                stop=not diag,
                                skip_group_check=True,
                            )
                        off += 512
                    if causal and hv == 0:
                        for s, (ki, (h0, hl)) in zip(s_list, batch):
                            nc.tensor.matmul(
                                s[:, 0:128], id_b[:], tri_b[:],
                                start=False, stop=True, skip_group_check=True,
                            )
                    if hv == 0:
                        # PE filler: AV of previous pair while exps run
                        if prevB is not None:
                            for ki_p in prevB[0]:
                                e_p_, vpt_p = prevB[1][ki_p]
                                emit_av2(ki_p, e_p_, vpt_p)
                        # oproj jobs once the B gathers have landed
                        if causal and t == 4:
                            oproj(0)
                        if causal and t == 6:
                            oproj(1)
                    for s, (ki, (h0, hl)) in zip(s_list, batch):
                        et = e_p.tile([128, 1024], bf16, tag="e")
                        acc = st_p.tile([128, 1], f32, tag="acc")
                        nc.scalar.activation(
                            et[:, 0:hl], s[:, 0:hl], EXP,
                            scale=SCALE, accum_out=acc[:],
                        )
                        ets[ki].append((h0, hl, et))
                        accs[ki].append(acc)
                cur = {}
                for ki in kis:
                    vpt = make_vpt(accs[ki], ki, 2)
                    cur[ki] = (ets[ki], vpt)
                if causal and t >= 2 and t % 2 == 0:
                    flushB(t // 2 - 1, av2)
                prevB = (kis, cur)
            for ki_p in prevB[0]:
                e_p_, vpt_p = prevB[1][ki_p]
                emit_av2(ki_p, e_p_, vpt_p)
            if causal:
                flushB(3, av2)
                oproj(2)
                oproj(3)
            else:
                for f in range(4):
                    flushB(f, av2)
                for f in range(4):
                    oproj(f)
            psO_ctx.__exit__(None, None, None)
            psB_ctx.__exit__(None, None, None)
    nc.compile()
    return nc


def _shards(x, mask, W_in, b_in, W_out, b_out):
    """Build per-core input maps (host-side sharding / layout prep)."""
    tri_np = np.where(
        np.arange(128)[None, :] < np.arange(128)[:, None], np.float32(NEG), 0.0
    ).astype(np.float32)
    # split-AllGather row order: rank pairs (h=3r,3r+1) then solos (h=3r+2)
    head_order = [0, 1, 3, 4, 6, 7, 9, 10, 2, 5, 8, 11]
    row_perm = np.concatenate([np.arange(h * 64, (h + 1) * 64) for h in head_order])
    in_maps = []
    for c in range(NCORES):
        b = c // GROUPS
        g = c % GROUPS
        hs = [3 * g, 3 * g + 1, 3 * g + 2]
        qc = [W_in[:, 64 * h : 64 * (h + 1)] for h in hs]
        kc = [W_in[:, D + 64 * h : D + 64 * (h + 1)] for h in hs]
        vc = W_in[:, 2 * D + 64 * hs[0] : 2 * D + 64 * (hs[2] + 1)]
        qb = [b_in[64 * h : 64 * (h + 1)] for h in hs]
        kb = [b_in[D + 64 * h : D + 64 * (h + 1)] for h in hs]
        vb = b_in[2 * D + 64 * hs[0] : 2 * D + 64 * (hs[2] + 1)]
        wqk = np.concatenate(
            [kc[0], kc[1], qc[0], qc[1], qc[2], kc[2]], axis=1
        ).astype(np.float32)
        bqk = np.concatenate([kb[0], kb[1], qb[0], qb[1], qb[2], kb[2]])
        in_maps.append(
            {
                "xT": np.ascontiguousarray(x[b].T, dtype=np.float32),
                "wqk": np.ascontiguousarray(wqk),
                "wv": np.ascontiguousarray(vc, dtype=np.float32),
                "bqkc": np.ascontiguousarray(bqk[:, None], dtype=np.float32),
                "bv": np.ascontiguousarray(vb[None, :], dtype=np.float32),
                "wout": np.ascontiguousarray(
                    W_out[row_perm, EPG * g : EPG * (g + 1)], dtype=np.float32
                ),
                "boutc": np.ascontiguousarray(
                    b_out[EPG * g : EPG * (g + 1), None], dtype=np.float32
                ),
                "tri": tri_np,
                "ident": np.eye(128, dtype=np.float32),
            }
        )
    return in_maps


def _numpy_ref(x, mask, W_in, b_in, W_out, b_out):
    qkv = x @ W_in + b_in
    q, k, v = np.split(qkv, 3, axis=2)
    q = q.reshape(B, S, H, DH).transpose(0, 2, 1, 3)
    k = k.reshape(B, S, H, DH).transpose(0, 2, 1, 3)
    v = v.reshape(B, S, H, DH).transpose(0, 2, 1, 3)
    attn = np.einsum("bhqd,bhkd->bhqk", q, k) / np.sqrt(np.float32(D))
    attn = np.where(mask == 0, -np.inf, attn)
    attn = attn - attn.max(axis=-2, keepdims=True)
    e = np.exp(attn)
    attn = e / e.sum(axis=-2, keepdims=True)
    out = np.einsum("bhqk,bhkd->bhqd", attn, v)
    out = out.transpose(0, 2, 1, 3).reshape(B, S, D)
    return (out @ W_out + b_out).astype(np.float32)


def _run(inputs, trace=False):
    from concourse.bass_utils import run_bass_kernel_spmd

    x = np.asarray(inputs["x"], dtype=np.float32)
    mask = np.asarray(inputs["mask"])
    W_in = np.asarray(inputs["W_in"], dtype=np.float32)
    b_in = np.asarray(inputs["b_in"], dtype=np.float32)
    W_out = np.asarray(inputs["W_out"], dtype=np.float32)
    b_out = np.asarray(inputs["b_out"], dtype=np.float32)

    m2 = np.asarray(mask).reshape(S, S)
    if np.array_equal(m2, np.tril(np.ones((S, S), m2.dtype))):
        causal = True
    elif np.array_equal(m2, np.ones((S, S), m2.dtype)):
        causal = False
    else:
        return _numpy_ref(x, mask, W_in, b_in, W_out, b_out), None

    key = ("nc", causal)
    if key not in _cache:
        _cache[key] = _build(causal)
    nc = _cache[key]

    in_maps = _shards(x, mask, W_in, b_in, W_out, b_out)
    res = run_bass_kernel_spmd(nc, in_maps, core_ids=list(range(NCORES)), trace=trace)

    full = np.empty((B, S, D), dtype=np.float32)
    for c in range(NCORES):
        b, g = c // GROUPS, c % GROUPS
        full[b, :, EPG * g : EPG * (g + 1)] = res.results[c]["out"].T
    return full, res


def kernel(**inputs) -> np.ndarray:
    out, _ = _run(inputs, trace=False)
    return out


# revision 9
# speedup vs baseline: 1.0772x; 1.0772x over previous
"""Trainium2 Bass kernel for nn_AttentionHead (B=2, S=2048, D=768, H=12).

Sharding: 8 cores = 2 batches x 4 head-groups (3 heads each).
Per core: QKV projection for its heads (transposed layout), causal
attention with softmax over the QUERY axis (reference peculiarity:
softmax dim=-2, scaled by sqrt(d_model)), AllGather of per-head outputs
within each batch's 4-core group, then a column-slice of the output
projection.  Host only slices / transposes / concatenates.

v2 structure (vs v1):
  - DMA priority order (wqk, then x in 512-col chunks, wv, rest) with
    PE warm-up matmuls covering the load phase and QKV chunks starting
    as soon as the first x columns land -- keeps the PE HAM clock warm.
  - waveA (heads 0,1) runs FIRST so its per-512-block AllGathers fire
    early; waveB (head 2) runs second with the output projection
    interleaved per block as the B gathers complete.
  - Software pipelining: AV(ki-1) is emitted between the score halves
    of ki so the PE has work while ScalarE exps; v_tiles/qkv2 chunks
    fill remaining PE slack.
  - The causal diagonal mask is applied on the PE (identity @ triT
    accumulated into the score PSUM) instead of a DVE add.
  - waveB AV packs the two 1024-column halves into PE column groups
    (partitions 0-63 / 64-127), halving its AV wall time and shrinking
    the accumulator to 2 PSUM banks.
"""

import math

import numpy as np

B, S, D, H, DH = 2, 2048, 768, 12, 64
NCORES = 8
GROUPS = 4  # head-groups per batch
HPG = 3  # heads per group
EPG = HPG * DH  # 192
SCALE = 1.0 / math.sqrt(D)
NEG = -1.0e30

_cache = {}


def _build(causal: bool):
    import contextlib

    import concourse.bacc as bacc
    import concourse.mybir as mybir
    from concourse import tile

    f32 = mybir.dt.float32
    bf16 = mybir.dt.bfloat16
    EXP = mybir.ActivationFunctionType.Exp

    nc = bacc.Bacc("TRN2", target_bir_lowering=False, debug=False, num_devices=NCORES)

    xT = nc.dram_tensor("xT", [D, S], f32, kind="ExternalInput")
    wqk = nc.dram_tensor("wqk", [D, 384], f32, kind="ExternalInput")
    wv = nc.dram_tensor("wv", [D, EPG], f32, kind="ExternalInput")
    bqkc = nc.dram_tensor("bqkc", [384, 1], f32, kind="ExternalInput")
    bv = nc.dram_tensor("bv", [1, EPG], f32, kind="ExternalInput")
    wout = nc.dram_tensor("wout", [D, EPG], f32, kind="ExternalInput")
    boutc = nc.dram_tensor("boutc", [EPG, 1], f32, kind="ExternalInput")
    tri = nc.dram_tensor("tri", [128, 128], f32, kind="ExternalInput")
    ident = nc.dram_tensor("ident", [128, 128], f32, kind="ExternalInput")
    out = nc.dram_tensor("out", [EPG, S], f32, kind="ExternalOutput")

    ag_inA = [nc.dram_tensor(f"ag_inA{f}", [128, 512], bf16) for f in range(4)]
    ag_outA = [nc.dram_tensor(f"ag_outA{f}", [512, 512], bf16) for f in range(4)]
    ag_inB = [nc.dram_tensor(f"ag_inB{f}", [64, 512], bf16) for f in range(4)]
    ag_outB = [nc.dram_tensor(f"ag_outB{f}", [256, 512], bf16) for f in range(4)]

    groups = [[0, 1, 2, 3], [4, 5, 6, 7]]

    with tile.TileContext(nc) as tc:
        with contextlib.ExitStack() as ctx:
            const_p = ctx.enter_context(tc.tile_pool(name="const", bufs=1))
            w_p = ctx.enter_context(tc.tile_pool(name="w", bufs=6))
            xs_p = ctx.enter_context(tc.tile_pool(name="xs", bufs=6))
            xt_p = ctx.enter_context(tc.tile_pool(name="xt", bufs=1))
            qk_p = ctx.enter_context(tc.tile_pool(name="qk", bufs=1))
            v_p = ctx.enter_context(tc.tile_pool(name="v", bufs=1))
            e_p = ctx.enter_context(tc.tile_pool(name="e", bufs=10))
            st_p = ctx.enter_context(tc.tile_pool(name="stat", bufs=24))
            vp_p = ctx.enter_context(tc.tile_pool(name="vp", bufs=6))
            atn_p = ctx.enter_context(tc.tile_pool(name="atn", bufs=1))
            ag_p = ctx.enter_context(tc.tile_pool(name="ag", bufs=1))
            o_p = ctx.enter_context(tc.tile_pool(name="oT", bufs=2))
            psS = ctx.enter_context(tc.tile_pool(name="psS", bufs=2, space="PSUM"))

            # ---- constants ----
            ones_f = const_p.tile([1, 512], f32)
            nc.vector.memset(ones_f[:], 1.0)
            ones = const_p.tile([1, 512], bf16)
            nc.vector.tensor_copy(ones[:], ones_f[:])
            tri_f = const_p.tile([128, 128], f32)
            nc.sync.dma_start(tri_f[:], tri[:, :])
            tri_b = const_p.tile([128, 128], bf16)
            nc.vector.tensor_copy(tri_b[:], tri_f[:])
            id_f = const_p.tile([128, 128], f32)
            nc.sync.dma_start(id_f[:], ident[:, :])
            id_b = const_p.tile([128, 128], bf16)
            nc.vector.tensor_copy(id_b[:], id_f[:])

            bqk_c = const_p.tile([128, 3], f32)
            nc.sync.dma_start(bqk_c[:], bqkc[:, :].rearrange("(c p) o -> p (c o)", p=128))
            bout_c = const_p.tile([128, 2], f32)
            nc.sync.dma_start(bout_c[0:64, 1:2], boutc[128:EPG, :])
            nc.sync.dma_start(bout_c[:, 0:1], boutc[0:128, :])
            bv_f = const_p.tile([1, EPG], f32)
            nc.sync.dma_start(bv_f[:], bv[:, :])
            bv_t = const_p.tile([1, EPG], bf16)
            nc.vector.tensor_copy(bv_t[:], bv_f[:])

            # ---- PE warm-up: dummy matmuls while the first DMAs land ----
            # bufs=1 WAW-serializes them (~770ns each) which stretches a few
            # matmuls across the whole x-load window, keeping HAM warm.
            warm_ctx = tc.tile_pool(name="psW", bufs=1, space="PSUM")
            psW = warm_ctx.__enter__()
            warm_in = const_p.tile([128, 512], bf16)
            nc.vector.memset(warm_in[:], 0.0)
            for wi in range(10):
                wps = psW.tile([128, 512], f32, tag="warm")
                nc.tensor.matmul(
                    wps[:], id_b[:], warm_in[:],
                    start=True, stop=True, skip_group_check=True,
                )

            # ---- staged weight/x DMAs in priority order + casts ----
            # wqk first (needed by the first qkv chunks), then x column
            # chunks striped across the scalar/gpsimd/sync DMA queues so
            # the transfers overlap; wv/wout later.
            wqk_t, wv_t, wout_t = [], [], []
            for dt_i in range(6):
                wf = w_p.tile([128, 384], f32, tag="wstg")
                nc.sync.dma_start(wf[:], wqk[dt_i * 128 : (dt_i + 1) * 128, :])
                wt = w_p.tile([128, 384], bf16, tag="wqk")
                nc.vector.tensor_copy(wt[:], wf[:])
                wqk_t.append(wt)

            xt_t = [
                xt_p.tile([128, S], bf16, tag=f"xt{i}", name=f"xt{i}")
                for i in range(6)
            ]

            def x_chunk(dt_i, sc):
                xs = xs_p.tile([128, 512], f32, tag="xstg", bufs=12)
                dmaq = (nc.scalar, nc.gpsimd, nc.sync)[(sc * 6 + dt_i) % 3]
                dmaq.dma_start(
                    xs[:], xT[dt_i * 128 : (dt_i + 1) * 128, sc * 512 : (sc + 1) * 512]
                )
                dst = xt_t[dt_i][:, sc * 512 : (sc + 1) * 512]
                if sc == 3:
                    nc.scalar.copy(dst, xs[:])
                elif sc == 1:
                    nc.gpsimd.tensor_copy(dst, xs[:])
                else:
                    nc.vector.tensor_copy(dst, xs[:])

            # qkv chunk c (128 rows of [k0k1|q0q1|q2k2]) for seq cols
            # [sc*512, (sc+1)*512); bias folded into the PSUM evacuation.
            def qkv_chunk(dst, c, sc):
                pt = psS.tile([128, 1024], f32, tag="strip")
                for dt_i in range(6):
                    nc.tensor.matmul(
                        pt[:, 0:512],
                        wqk_t[dt_i][:, c * 128 : (c + 1) * 128],
                        xt_t[dt_i][:, sc * 512 : (sc + 1) * 512],
                        start=(dt_i == 0), stop=(dt_i == 5),
                    )
                nc.vector.tensor_scalar_add(
                    dst[:, sc * 512 : (sc + 1) * 512], pt[:, 0:512], bqk_c[:, c : c + 1]
                )

            vnat = v_p.tile([128, 16 * EPG], bf16)

            def v_tile(st_i):
                ptf = psS.tile([128, 1024], f32, tag="strip")
                p = ptf[:, 0:EPG]
                nc.tensor.matmul(p, ones[:, 0:128], bv_t[:], start=True, stop=False)
                for dt_i in range(6):
                    nc.tensor.matmul(
                        p,
                        xt_t[dt_i][:, st_i * 128 : (st_i + 1) * 128],
                        wv_t[dt_i][:],
                        start=False, stop=(dt_i == 5),
                    )
                nc.vector.tensor_copy(vnat[:, st_i * EPG : (st_i + 1) * EPG], p)

            k01 = qk_p.tile([128, S], bf16, tag="k01")
            q01 = qk_p.tile([128, S], bf16, tag="q01")
            qk2 = qk_p.tile([128, S], bf16, tag="qk2")
            k2 = qk_p.tile([64, S], bf16, tag="k2")
            q2d = qk_p.tile([128, S], bf16, tag="q2d")

            # phase 1: all x chunk DMAs upfront (striped queues), then wv,
            # qkv01 chunks in sc order, wout, first v_tiles.
            for sc in range(4):
                for dt_i in range(6):
                    x_chunk(dt_i, sc)
            for dt_i in range(6):
                vf = w_p.tile([128, EPG], f32, tag="wvstg")
                nc.sync.dma_start(vf[:], wv[dt_i * 128 : (dt_i + 1) * 128, :])
                vt = w_p.tile([128, EPG], bf16, tag="wv")
                nc.gpsimd.tensor_copy(vt[:], vf[:])
                wv_t.append(vt)
            for sc in range(4):
                qkv_chunk(k01, 0, sc)
                qkv_chunk(q01, 1, sc)
                if sc == 2:
                    # wout last (needed only at oproj time)
                    for dt_i in range(6):
                        wos = w_p.tile([128, EPG], f32, tag="wostg")
                        nc.sync.dma_start(wos[:], wout[dt_i * 128 : (dt_i + 1) * 128, :])
                        wo = w_p.tile([128, EPG], bf16, tag="wout")
                        nc.gpsimd.tensor_copy(wo[:], wos[:])
                        wout_t.append(wo)
            v_tile(0)
            v_tile(1)
            v_tile(2)
            warm_ctx.__exit__(None, None, None)

            def halves_of(ki):
                q0 = 128 * ki if causal else 0
                L = S - q0
                hs = [(q0, min(L, 1024))]
                if L > 1024:
                    hs.append((q0 + 1024, L - 1024))
                return hs

            def make_vpt(accs, ki, head):
                rcp = st_p.tile([128, 1], f32, tag="rcp")
                if len(accs) == 2:
                    ssum = st_p.tile([128, 1], f32, tag="ssum")
                    nc.vector.tensor_add(ssum[:], accs[0][:], accs[1][:])
                    nc.vector.reciprocal(rcp[:], ssum[:])
                else:
                    nc.vector.reciprocal(rcp[:], accs[0][:])
                vpt = vp_p.tile([128, 64], bf16, tag="vp")
                nc.vector.tensor_scalar_mul(
                    vpt[:],
                    vnat[:, ki * EPG + head * 64 : ki * EPG + (head + 1) * 64],
                    rcp[:],
                )
                return vpt

            agA_t = [
                ag_p.tile([128, S], bf16, tag=f"agA{i}", name=f"agA{i}")
                for i in range(4)
            ]
            agB_t = [
                ag_p.tile([128, S], bf16, tag=f"agB{i}", name=f"agB{i}")
                for i in range(2)
            ]

            atnA = atn_p.tile([128, S], bf16, tag="atnA")
            atn2 = atn_p.tile([64, S], bf16, tag="atn2")

            def flushA(f, avA):
                cols = slice(512 * f, 512 * (f + 1))
                nc.vector.tensor_copy(atnA[:, cols], avA[:, cols])
                nc.sync.dma_start(ag_inA[f][:, :], atnA[:, cols])
                nc.gpsimd.collective_compute(
                    "AllGather",
                    mybir.AluOpType.bypass,
                    replica_groups=groups,
                    ins=[ag_inA[f].ap().opt()],
                    outs=[ag_outA[f].ap().opt()],
                )
                for dt_i in range(4):
                    nc.sync.dma_start(
                        agA_t[dt_i][:, cols],
                        ag_outA[f][dt_i * 128 : (dt_i + 1) * 128, :],
                    )

            def flushB(f, av2):
                cols = slice(512 * f, 512 * (f + 1))
                if causal:
                    src = av2[0:64, cols] if f < 2 else av2[64:128, slice(512 * (f - 2), 512 * (f - 1))]
                else:
                    src = av2[0:64, cols] if f < 2 else av2[64:128, slice(512 * (f - 2), 512 * (f - 1))]
                nc.vector.tensor_copy(atn2[:, cols], src)
                nc.sync.dma_start(ag_inB[f][:, :], atn2[:, cols])
                nc.gpsimd.collective_compute(
                    "AllGather",
                    mybir.AluOpType.bypass,
                    replica_groups=groups,
                    ins=[ag_inB[f].ap().opt()],
                    outs=[ag_outB[f].ap().opt()],
                )
                for dt_i in range(2):
                    nc.sync.dma_start(
                        agB_t[dt_i][:, cols],
                        ag_outB[f][dt_i * 128 : (dt_i + 1) * 128, :],
                    )

            # ---- wave A: heads 0+1, row/col-group paired, pipelined ----
            psA_ctx = tc.tile_pool(name="psA", bufs=1, space="PSUM")
            psA = psA_ctx.__enter__()
            avA = psA.tile([128, S], f32, tag="avA")

            def emit_avA(ki, ets, vpts):
                for hv, (h0, hl, et0, et1) in enumerate(ets):
                    off = 0
                    while off < hl:
                        n = min(512, hl - off)
                        for hi, et in ((0, et0), (1, et1)):
                            nc.tensor.matmul(
                                avA[64 * hi : 64 * hi + 64, h0 + off : h0 + off + n],
                                vpts[hi][:],
                                et[:, off : off + n],
                                start=(ki == 0),
                                stop=(ki == 15),
                                skip_group_check=True,
                            )
                        off += n

            prev = None  # (ki, ets, vpts)
            for ki in range(16):
                hs = halves_of(ki)
                cur_ets = []
                accs = {0: [], 1: []}
                for hv, (h0, hl) in enumerate(hs):
                    s0 = psS.tile([128, 1024], f32, tag="strip")
                    s1 = psS.tile([128, 1024], f32, tag="strip")
                    off = 0
                    while off < hl:
                        n = min(512, hl - off)
                        diag = causal and hv == 0 and off == 0
                        for hi, s in ((0, s0), (1, s1)):
                            nc.tensor.matmul(
                                s[:, off : off + n],
                                k01[64 * hi : 64 * hi + 64, ki * 128 : (ki + 1) * 128],
                                q01[64 * hi : 64 * hi + 64, h0 + off : h0 + off + n],
                                start=True,
                                stop=not diag,
                                skip_group_check=True,
                            )
                        if diag:
                            for s in (s0, s1):
                                nc.tensor.matmul(
                                    s[:, 0:128], id_b[:], tri_b[:],
                                    start=False, stop=True, skip_group_check=True,
                                )
                        off += n
                    if hv == 0:
                        # PE filler while ScalarE exps this ki: AV of ki-1,
                        # then one interleave job.
                        if prev is not None:
                            emit_avA(prev[0], prev[1], prev[2])
                        if ki + 3 <= 15:
                            v_tile(ki + 3)
                        if ki >= 8 and ki % 2 == 0:
                            sc = (ki - 8) // 2
                            qkv_chunk(qk2, 2, sc)
                            cols = slice(sc * 512, (sc + 1) * 512)
                            nc.gpsimd.dma_start(k2[:, cols], qk2[64:128, cols])
                            nc.gpsimd.dma_start(q2d[64:128, cols], qk2[0:64, cols])
                    ets_half = []
                    for hi, s in ((0, s0), (1, s1)):
                        et = e_p.tile([128, 1024], bf16, tag="e")
                        acc = st_p.tile([128, 1], f32, tag="acc")
                        nc.scalar.activation(
                            et[:, 0:hl], s[:, 0:hl], EXP,
                            scale=SCALE, accum_out=acc[:],
                        )
                        ets_half.append(et)
                        accs[hi].append(acc)
                    cur_ets.append((h0, hl, ets_half[0], ets_half[1]))
                vpts = [make_vpt(accs[hi], ki, hi) for hi in range(2)]
                # flush completed 512-col blocks (block f done after AV(4f+3),
                # which was emitted during ki=4f+4's hv0 above)
                if causal and ki >= 4 and (ki % 4) == 0:
                    flushA(ki // 4 - 1, avA)
                prev = (ki, cur_ets, vpts)
            emit_avA(prev[0], prev[1], prev[2])
            if causal:
                flushA(3, avA)
            else:
                for f in range(4):
                    flushA(f, avA)
            psA_ctx.__exit__(None, None, None)

            # ---- wave B: head 2, ki pairs in row groups, col-half-packed AV
            psB_ctx = tc.tile_pool(name="psB", bufs=1, space="PSUM")
            psB = psB_ctx.__enter__()
            psO_ctx = tc.tile_pool(name="psO", bufs=2, space="PSUM")
            psO = psO_ctx.__enter__()
            av2 = psB.tile([128, 1024], f32, tag="av2")

            def emit_av2(ki, ets, vpt):
                # col-half packing: abs cols [0,1024) -> av2[0:64],
                # [1024,2048) -> av2[64:128, col-1024]; chunks may not
                # cross the 1024 boundary.
                for (h0, hl, et) in ets:
                    off = 0
                    while off < hl:
                        col = h0 + off
                        n = min(512, hl - off)
                        if col < 1024:
                            n = min(n, 1024 - col)
                            dst = av2[0:64, col : col + n]
                        else:
                            dst = av2[64:128, col - 1024 : col - 1024 + n]
                        nc.tensor.matmul(
                            dst,
                            vpt[:],
                            et[:, off : off + n],
                            start=(ki == 0),
                            stop=(ki == 15),
                            skip_group_check=True,
                        )
                        off += n

            def oproj(f):
                cols = slice(512 * f, 512 * (f + 1))
                src_t = agA_t + agB_t
                for mc, (m0, mw) in enumerate([(0, 128), (128, 64)]):
                    pt = psO.tile([128, 512], f32, tag="po")
                    for dt_i in range(6):
                        nc.tensor.matmul(
                            pt[0:mw, :],
                            wout_t[dt_i][:, m0 : m0 + mw],
                            src_t[dt_i][:, cols],
                            start=(dt_i == 0), stop=(dt_i == 5),
                        )
                    oT = o_p.tile([128, 512], f32, tag="oT")
                    nc.vector.tensor_scalar_add(
                        oT[0:mw, :], pt[0:mw, :], bout_c[0:mw, mc : mc + 1]
                    )
                    nc.sync.dma_start(out[m0 : m0 + mw, cols], oT[0:mw, :])

            srcs = {}
            for ki in range(16):
                if ki % 2 == 0:
                    srcs[ki] = (k2, 0, qk2, 0)
                else:
                    srcs[ki] = (qk2, 64, q2d, 64)

            prevB = None  # (kis, {ki: (ets, vpt)})
            for t in range(8):
                kis = (2 * t, 2 * t + 1)
                ets = {ki: [] for ki in kis}
                accs = {ki: [] for ki in kis}
                maxhv = max(len(halves_of(ki)) for ki in kis)
                for hv in range(maxhv):
                    batch = []
                    for ki in kis:
                        hsk = halves_of(ki)
                        if hv < len(hsk):
                            batch.append((ki, hsk[hv]))
                    s_list = [
                        psS.tile([128, 1024], f32, tag="strip", name="sB")
                        for _ in batch
                    ]
                    maxhl = max(hl for _, (_, hl) in batch)
                    off = 0
                    while off < maxhl:
                        for s, (ki, (h0, hl)) in zip(s_list, batch):
                            if off >= hl:
                                continue
                            n = min(512, hl - off)
                            kT, kb, qT, qb = srcs[ki]
                            diag = causal and hv == 0 and off == 0
                            nc.tensor.matmul(
                                s[:, off : off + n],
                                kT[kb : kb + 64, ki * 128 : (ki + 1) * 128],
                                qT[qb : qb + 64, h0 + off : h0 + off + n],
                                start=True,
                                stop=not diag,
                                skip_group_check=True,
                            )
                        off += 512
                    if causal and hv == 0:
                        for s, (ki, (h0, hl)) in zip(s_list, batch):
                            nc.tensor.matmul(
                                s[:, 0:128], id_b[:], tri_b[:],
                                start=False, stop=True, skip_group_check=True,
                            )
                    if hv == 0:
                        # PE filler: AV of previous pair while exps run
                        if prevB is not None:
                            for ki_p in prevB[0]:
                                e_p_, vpt_p = prevB[1][ki_p]
                                emit_av2(ki_p, e_p_, vpt_p)
                        # oproj jobs once the B gathers have landed
                        if causal and t == 4:
                            oproj(0)
                        if causal and t == 6:
                            oproj(1)
                    for s, (ki, (h0, hl)) in zip(s_list, batch):
                        et = e_p.tile([128, 1024], bf16, tag="e")
                        acc = st_p.tile([128, 1], f32, tag="acc")
                        nc.scalar.activation(
                            et[:, 0:hl], s[:, 0:hl], EXP,
                            scale=SCALE, accum_out=acc[:],
                        )
                        ets[ki].append((h0, hl, et))
                        accs[ki].append(acc)
                cur = {}
                for ki in kis:
                    vpt = make_vpt(accs[ki], ki, 2)
                    cur[ki] = (ets[ki], vpt)
                if causal and t >= 2 and t % 2 == 0:
                    flushB(t // 2 - 1, av2)
                prevB = (kis, cur)
            for ki_p in prevB[0]:
                e_p_, vpt_p = prevB[1][ki_p]
                emit_av2(ki_p, e_p_, vpt_p)
            if causal:
                flushB(3, av2)
                oproj(2)
                oproj(3)
            else:
                for f in range(4):
                    flushB(f, av2)
                for f in range(4):
                    oproj(f)
            psO_ctx.__exit__(None, None, None)
            psB_ctx.__exit__(None, None, None)
    nc.compile()
    return nc


def _shards(x, mask, W_in, b_in, W_out, b_out):
    """Build per-core input maps (host-side sharding / layout prep)."""
    tri_np = np.where(
        np.arange(128)[None, :] < np.arange(128)[:, None], np.float32(NEG), 0.0
    ).astype(np.float32)
    # split-AllGather row order: rank pairs (h=3r,3r+1) then solos (h=3r+2)
    head_order = [0, 1, 3, 4, 6, 7, 9, 10, 2, 5, 8, 11]
    row_perm = np.concatenate([np.arange(h * 64, (h + 1) * 64) for h in head_order])
    in_maps = []
    for c in range(NCORES):
        b = c // GROUPS
        g = c % GROUPS
        hs = [3 * g, 3 * g + 1, 3 * g + 2]
        qc = [W_in[:, 64 * h : 64 * (h + 1)] for h in hs]
        kc = [W_in[:, D + 64 * h : D + 64 * (h + 1)] for h in hs]
        vc = W_in[:, 2 * D + 64 * hs[0] : 2 * D + 64 * (hs[2] + 1)]
        qb = [b_in[64 * h : 64 * (h + 1)] for h in hs]
        kb = [b_in[D + 64 * h : D + 64 * (h + 1)] for h in hs]
        vb = b_in[2 * D + 64 * hs[0] : 2 * D + 64 * (hs[2] + 1)]
        wqk = np.concatenate(
            [kc[0], kc[1], qc[0], qc[1], qc[2], kc[2]], axis=1
        ).astype(np.float32)
        bqk = np.concatenate([kb[0], kb[1], qb[0], qb[1], qb[2], kb[2]])
        in_maps.append(
            {
                "xT": np.ascontiguousarray(x[b].T, dtype=np.float32),
                "wqk": np.ascontiguousarray(wqk),
                "wv": np.ascontiguousarray(vc, dtype=np.float32),
                "bqkc": np.ascontiguousarray(bqk[:, None], dtype=np.float32),
                "bv": np.ascontiguousarray(vb[None, :], dtype=np.float32),
                "wout": np.ascontiguousarray(
                    W_out[row_perm, EPG * g : EPG * (g + 1)], dtype=np.float32
                ),
                "boutc": np.ascontiguousarray(
                    b_out[EPG * g : EPG * (g + 1), None], dtype=np.float32
                ),
                "tri": tri_np,
                "ident": np.eye(128, dtype=np.float32),
            }
        )
    return in_maps


def _numpy_ref(x, mask, W_in, b_in, W_out, b_out):
    qkv = x @ W_in + b_in
    q, k, v = np.split(qkv, 3, axis=2)
    q = q.reshape(B, S, H, DH).transpose(0, 2, 1, 3)
    k = k.reshape(B, S, H, DH).transpose(0, 2, 1, 3)
    v = v.reshape(B, S, H, DH).transpose(0, 2, 1, 3)
    attn = np.einsum("bhqd,bhkd->bhqk", q, k) / np.sqrt(np.float32(D))
    attn = np.where(mask == 0, -np.inf, attn)
    attn = attn - attn.max(axis=-2, keepdims=True)
    e = np.exp(attn)
    attn = e / e.sum(axis=-2, keepdims=True)
    out = np.einsum("bhqk,bhkd->bhqd", attn, v)
    out = out.transpose(0, 2, 1, 3).reshape(B, S, D)
    return (out @ W_out + b_out).astype(np.float32)


def _run(inputs, trace=False):
    from concourse.bass_utils import run_bass_kernel_spmd

    x = np.asarray(inputs["x"], dtype=np.float32)
    mask = np.asarray(inputs["mask"])
    W_in = np.asarray(inputs["W_in"], dtype=np.float32)
    b_in = np.asarray(inputs["b_in"], dtype=np.float32)
    W_out = np.asarray(inputs["W_out"], dtype=np.float32)
    b_out = np.asarray(inputs["b_out"], dtype=np.float32)

    m2 = np.asarray(mask).reshape(S, S)
    if np.array_equal(m2, np.tril(np.ones((S, S), m2.dtype))):
        causal = True
    elif np.array_equal(m2, np.ones((S, S), m2.dtype)):
        causal = False
    else:
        return _numpy_ref(x, mask, W_in, b_in, W_out, b_out), None

    key = ("nc", causal)
    if key not in _cache:
        _cache[key] = _build(causal)
    nc = _cache[key]

    in_maps = _shards(x, mask, W_in, b_in, W_out, b_out)
    res = run_bass_kernel_spmd(nc, in_maps, core_ids=list(range(NCORES)), trace=trace)

    full = np.empty((B, S, D), dtype=np.float32)
    for c in range(NCORES):
        b, g = c // GROUPS, c % GROUPS
        full[b, :, EPG * g : EPG * (g + 1)] = res.results[c]["out"].T
    return full, res


def kernel(**inputs) -> np.ndarray:
    out, _ = _run(inputs, trace=False)
    return out


# revision 10
# speedup vs baseline: 1.0830x; 1.0054x over previous
"""Trainium2 Bass kernel for nn_AttentionHead (B=2, S=2048, D=768, H=12).

Sharding: 8 cores = 2 batches x 4 head-groups (3 heads each).
Per core: QKV projection for its heads (transposed layout), causal
attention with softmax over the QUERY axis (reference peculiarity:
softmax dim=-2, scaled by sqrt(d_model)), AllGather of per-head outputs
within each batch's 4-core group, then a column-slice of the output
projection.  Host only slices / transposes / concatenates.

v2 structure (vs v1):
  - DMA priority order (wqk, then x in 512-col chunks, wv, rest) with
    PE warm-up matmuls covering the load phase and QKV chunks starting
    as soon as the first x columns land -- keeps the PE HAM clock warm.
  - waveA (heads 0,1) runs FIRST so its per-512-block AllGathers fire
    early; waveB (head 2) runs second with the output projection
    interleaved per block as the B gathers complete.
  - Software pipelining: AV(ki-1) is emitted between the score halves
    of ki so the PE has work while ScalarE exps; v_tiles/qkv2 chunks
    fill remaining PE slack.
  - The causal diagonal mask is applied on the PE (identity @ triT
    accumulated into the score PSUM) instead of a DVE add.
  - waveB AV packs the two 1024-column halves into PE column groups
    (partitions 0-63 / 64-127), halving its AV wall time and shrinking
    the accumulator to 2 PSUM banks.
"""

import math

import numpy as np

B, S, D, H, DH = 2, 2048, 768, 12, 64
NCORES = 8
GROUPS = 4  # head-groups per batch
HPG = 3  # heads per group
EPG = HPG * DH  # 192
SCALE = 1.0 / math.sqrt(D)
NEG = -1.0e30

_cache = {}


def _build(causal: bool):
    import contextlib

    import concourse.bacc as bacc
    import concourse.mybir as mybir
    from concourse import tile

    f32 = mybir.dt.float32
    bf16 = mybir.dt.bfloat16
    EXP = mybir.ActivationFunctionType.Exp

    nc = bacc.Bacc("TRN2", target_bir_lowering=False, debug=False, num_devices=NCORES)

    xT = nc.dram_tensor("xT", [D, S], f32, kind="ExternalInput")
    wqk = nc.dram_tensor("wqk", [D, 384], f32, kind="ExternalInput")
    wv = nc.dram_tensor("wv", [D, EPG], f32, kind="ExternalInput")
    bqkc = nc.dram_tensor("bqkc", [384, 1], f32, kind="ExternalInput")
    bv = nc.dram_tensor("bv", [1, EPG], f32, kind="ExternalInput")
    wout = nc.dram_tensor("wout", [D, EPG], f32, kind="ExternalInput")
    boutc = nc.dram_tensor("boutc", [EPG, 1], f32, kind="ExternalInput")
    tri = nc.dram_tensor("tri", [128, 128], f32, kind="ExternalInput")
    ident = nc.dram_tensor("ident", [128, 128], f32, kind="ExternalInput")
    out = nc.dram_tensor("out", [EPG, S], f32, kind="ExternalOutput")

    ag_inA = [nc.dram_tensor(f"ag_inA{f}", [128, 512], bf16) for f in range(4)]
    ag_outA = [nc.dram_tensor(f"ag_outA{f}", [512, 512], bf16) for f in range(4)]
    ag_inB = [nc.dram_tensor(f"ag_inB{f}", [64, 512], bf16) for f in range(4)]
    ag_outB = [nc.dram_tensor(f"ag_outB{f}", [256, 512], bf16) for f in range(4)]

    groups = [[0, 1, 2, 3], [4, 5, 6, 7]]

    with tile.TileContext(nc) as tc:
        with contextlib.ExitStack() as ctx:
            const_p = ctx.enter_context(tc.tile_pool(name="const", bufs=1))
            w_p = ctx.enter_context(tc.tile_pool(name="w", bufs=6))
            xs_p = ctx.enter_context(tc.tile_pool(name="xs", bufs=6))
            xt_p = ctx.enter_context(tc.tile_pool(name="xt", bufs=1))
            qk_p = ctx.enter_context(tc.tile_pool(name="qk", bufs=1))
            v_p = ctx.enter_context(tc.tile_pool(name="v", bufs=1))
            e_p = ctx.enter_context(tc.tile_pool(name="e", bufs=10))
            st_p = ctx.enter_context(tc.tile_pool(name="stat", bufs=24))
            vp_p = ctx.enter_context(tc.tile_pool(name="vp", bufs=6))
            atn_p = ctx.enter_context(tc.tile_pool(name="atn", bufs=1))
            ag_p = ctx.enter_context(tc.tile_pool(name="ag", bufs=1))
            o_p = ctx.enter_context(tc.tile_pool(name="oT", bufs=2))
            psS = ctx.enter_context(tc.tile_pool(name="psS", bufs=2, space="PSUM"))

            # ---- constants ----
            ones_f = const_p.tile([1, 512], f32)
            nc.vector.memset(ones_f[:], 1.0)
            ones = const_p.tile([1, 512], bf16)
            nc.vector.tensor_copy(ones[:], ones_f[:])
            tri_f = const_p.tile([128, 128], f32)
            nc.sync.dma_start(tri_f[:], tri[:, :])
            tri_b = const_p.tile([128, 128], bf16)
            nc.vector.tensor_copy(tri_b[:], tri_f[:])
            id_f = const_p.tile([128, 128], f32)
            nc.sync.dma_start(id_f[:], ident[:, :])
            id_b = const_p.tile([128, 128], bf16)
            nc.vector.tensor_copy(id_b[:], id_f[:])

            bqk_c = const_p.tile([128, 3], f32)
            nc.sync.dma_start(bqk_c[:], bqkc[:, :].rearrange("(c p) o -> p (c o)", p=128))
            bout_c = const_p.tile([128, 2], f32)
            nc.sync.dma_start(bout_c[0:64, 1:2], boutc[128:EPG, :])
            nc.sync.dma_start(bout_c[:, 0:1], boutc[0:128, :])
            bv_f = const_p.tile([1, EPG], f32)
            nc.sync.dma_start(bv_f[:], bv[:, :])
            bv_t = const_p.tile([1, EPG], bf16)
            nc.vector.tensor_copy(bv_t[:], bv_f[:])

            # ---- PE warm-up: dummy matmuls while the first DMAs land ----
            # bufs=1 WAW-serializes them (~770ns each) which stretches a few
            # matmuls across the whole x-load window, keeping HAM warm.
            warm_ctx = tc.tile_pool(name="psW", bufs=1, space="PSUM")
            psW = warm_ctx.__enter__()
            warm_in = const_p.tile([128, 512], bf16)
            nc.vector.memset(warm_in[:], 0.0)
            for wi in range(10):
                wps = psW.tile([128, 512], f32, tag="warm")
                nc.tensor.matmul(
                    wps[:], id_b[:], warm_in[:],
                    start=True, stop=True, skip_group_check=True,
                )

            # ---- staged weight/x DMAs in priority order + casts ----
            # wqk first (needed by the first qkv chunks), then x column
            # chunks striped across the scalar/gpsimd/sync DMA queues so
            # the transfers overlap; wv/wout later.
            wqk_t, wv_t, wout_t = [], [], []
            for dt_i in range(6):
                wf = w_p.tile([128, 384], f32, tag="wstg")
                nc.sync.dma_start(wf[:], wqk[dt_i * 128 : (dt_i + 1) * 128, :])
                wt = w_p.tile([128, 384], bf16, tag="wqk")
                nc.vector.tensor_copy(wt[:], wf[:])
                wqk_t.append(wt)

            xt_t = [
                xt_p.tile([128, S], bf16, tag=f"xt{i}", name=f"xt{i}")
                for i in range(6)
            ]

            def x_chunk(dt_i, hf):
                xs = xs_p.tile([128, 1024], f32, tag="xstg", bufs=8)
                dmaq = nc.scalar if hf == 0 else nc.sync
                dmaq.dma_start(
                    xs[:],
                    xT[dt_i * 128 : (dt_i + 1) * 128, hf * 1024 : (hf + 1) * 1024],
                )
                dst = xt_t[dt_i][:, hf * 1024 : (hf + 1) * 1024]
                if hf == 0:
                    nc.vector.tensor_copy(dst, xs[:])
                else:
                    nc.scalar.copy(dst, xs[:])

            # qkv chunk c (128 rows of [k0k1|q0q1|q2k2]) for seq cols
            # [hf*1024, (hf+1)*1024); bias folded into the PSUM evacuation.
            def qkv_chunk(dst, c, hf):
                pt = psS.tile([128, 1024], f32, tag="strip")
                for off in (0, 512):
                    for dt_i in range(6):
                        nc.tensor.matmul(
                            pt[:, off : off + 512],
                            wqk_t[dt_i][:, c * 128 : (c + 1) * 128],
                            xt_t[dt_i][:, hf * 1024 + off : hf * 1024 + off + 512],
                            start=(dt_i == 0), stop=(dt_i == 5),
                        )
                nc.vector.tensor_scalar_add(
                    dst[:, hf * 1024 : (hf + 1) * 1024], pt[:], bqk_c[:, c : c + 1]
                )

            vnat = v_p.tile([128, 16 * EPG], bf16)

            def v_tile(st_i):
                ptf = psS.tile([128, 1024], f32, tag="strip")
                p = ptf[:, 0:EPG]
                nc.tensor.matmul(p, ones[:, 0:128], bv_t[:], start=True, stop=False)
                for dt_i in range(6):
                    nc.tensor.matmul(
                        p,
                        xt_t[dt_i][:, st_i * 128 : (st_i + 1) * 128],
                        wv_t[dt_i][:],
                        start=False, stop=(dt_i == 5),
                    )
                nc.vector.tensor_copy(vnat[:, st_i * EPG : (st_i + 1) * EPG], p)

            k01 = qk_p.tile([128, S], bf16, tag="k01")
            q01 = qk_p.tile([128, S], bf16, tag="q01")
            qk2 = qk_p.tile([128, S], bf16, tag="qk2")
            k2 = qk_p.tile([64, S], bf16, tag="k2")
            q2d = qk_p.tile([128, S], bf16, tag="q2d")

            # phase 1: x chunk DMAs upfront (halves split over the scalar
            # and sync queues), then wv, qkv01 chunks, wout, first v_tiles.
            for hf in range(2):
                for dt_i in range(6):
                    x_chunk(dt_i, hf)
            for dt_i in range(6):
                vf = w_p.tile([128, EPG], f32, tag="wvstg")
                nc.sync.dma_start(vf[:], wv[dt_i * 128 : (dt_i + 1) * 128, :])
                vt = w_p.tile([128, EPG], bf16, tag="wv")
                nc.gpsimd.tensor_copy(vt[:], vf[:])
                wv_t.append(vt)
            for hf in range(2):
                qkv_chunk(k01, 0, hf)
                qkv_chunk(q01, 1, hf)
                if hf == 0:
                    # wout last (needed only at oproj time)
                    for dt_i in range(6):
                        wos = w_p.tile([128, EPG], f32, tag="wostg")
                        nc.sync.dma_start(wos[:], wout[dt_i * 128 : (dt_i + 1) * 128, :])
                        wo = w_p.tile([128, EPG], bf16, tag="wout")
                        nc.gpsimd.tensor_copy(wo[:], wos[:])
                        wout_t.append(wo)
            v_tile(0)
            v_tile(1)
            v_tile(2)
            warm_ctx.__exit__(None, None, None)

            def halves_of(ki):
                q0 = 128 * ki if causal else 0
                L = S - q0
                hs = [(q0, min(L, 1024))]
                if L > 1024:
                    hs.append((q0 + 1024, L - 1024))
                return hs

            def make_vpt(accs, ki, head):
                rcp = st_p.tile([128, 1], f32, tag="rcp")
                if len(accs) == 2:
                    ssum = st_p.tile([128, 1], f32, tag="ssum")
                    nc.vector.tensor_add(ssum[:], accs[0][:], accs[1][:])
                    nc.vector.reciprocal(rcp[:], ssum[:])
                else:
                    nc.vector.reciprocal(rcp[:], accs[0][:])
                vpt = vp_p.tile([128, 64], bf16, tag="vp")
                nc.vector.tensor_scalar_mul(
                    vpt[:],
                    vnat[:, ki * EPG + head * 64 : ki * EPG + (head + 1) * 64],
                    rcp[:],
                )
                return vpt

            agA_t = [
                ag_p.tile([128, S], bf16, tag=f"agA{i}", name=f"agA{i}")
                for i in range(4)
            ]
            agB_t = [
                ag_p.tile([128, S], bf16, tag=f"agB{i}", name=f"agB{i}")
                for i in range(2)
            ]

            atnA = atn_p.tile([128, S], bf16, tag="atnA")
            atn2 = atn_p.tile([64, S], bf16, tag="atn2")

            def flushA(f, avA):
                cols = slice(512 * f, 512 * (f + 1))
                nc.vector.tensor_copy(atnA[:, cols], avA[:, cols])
                nc.sync.dma_start(ag_inA[f][:, :], atnA[:, cols])
                nc.gpsimd.collective_compute(
                    "AllGather",
                    mybir.AluOpType.bypass,
                    replica_groups=groups,
                    ins=[ag_inA[f].ap().opt()],
                    outs=[ag_outA[f].ap().opt()],
                )
                for dt_i in range(4):
                    nc.gpsimd.dma_start(
                        agA_t[dt_i][:, cols],
                        ag_outA[f][dt_i * 128 : (dt_i + 1) * 128, :],
                    )

            def flushB(f, av2):
                cols = slice(512 * f, 512 * (f + 1))
                if causal:
                    src = av2[0:64, cols] if f < 2 else av2[64:128, slice(512 * (f - 2), 512 * (f - 1))]
                else:
                    src = av2[0:64, cols] if f < 2 else av2[64:128, slice(512 * (f - 2), 512 * (f - 1))]
                nc.vector.tensor_copy(atn2[:, cols], src)
                nc.sync.dma_start(ag_inB[f][:, :], atn2[:, cols])
                nc.gpsimd.collective_compute(
                    "AllGather",
                    mybir.AluOpType.bypass,
                    replica_groups=groups,
                    ins=[ag_inB[f].ap().opt()],
                    outs=[ag_outB[f].ap().opt()],
                )
                for dt_i in range(2):
                    nc.gpsimd.dma_start(
                        agB_t[dt_i][:, cols],
                        ag_outB[f][dt_i * 128 : (dt_i + 1) * 128, :],
                    )

            # ---- wave A: heads 0+1, row/col-group paired, pipelined ----
            psA_ctx = tc.tile_pool(name="psA", bufs=1, space="PSUM")
            psA = psA_ctx.__enter__()
            avA = psA.tile([128, S], f32, tag="avA")

            def emit_avA(ki, ets, vpts):
                for hv, (h0, hl, et0, et1) in enumerate(ets):
                    off = 0
                    while off < hl:
                        n = min(512, hl - off)
                        for hi, et in ((0, et0), (1, et1)):
                            nc.tensor.matmul(
                                avA[64 * hi : 64 * hi + 64, h0 + off : h0 + off + n],
                                vpts[hi][:],
                                et[:, off : off + n],
                                start=(ki == 0),
                                stop=(ki == 15),
                                skip_group_check=True,
                            )
                        off += n

            prev = None  # (ki, ets, vpts)
            for ki in range(16):
                hs = halves_of(ki)
                cur_ets = []
                accs = {0: [], 1: []}
                for hv, (h0, hl) in enumerate(hs):
                    s0 = psS.tile([128, 1024], f32, tag="strip")
                    s1 = psS.tile([128, 1024], f32, tag="strip")
                    off = 0
                    while off < hl:
                        n = min(512, hl - off)
                        diag = causal and hv == 0 and off == 0
                        for hi, s in ((0, s0), (1, s1)):
                            nc.tensor.matmul(
                                s[:, off : off + n],
                                k01[64 * hi : 64 * hi + 64, ki * 128 : (ki + 1) * 128],
                                q01[64 * hi : 64 * hi + 64, h0 + off : h0 + off + n],
                                start=True,
                                stop=not diag,
                                skip_group_check=True,
                            )
                        if diag:
                            for s in (s0, s1):
                                nc.tensor.matmul(
                                    s[:, 0:128], id_b[:], tri_b[:],
                                    start=False, stop=True, skip_group_check=True,
                                )
                        off += n
                    if hv == 0:
                        # PE filler while ScalarE exps this ki: AV of ki-1,
                        # then one interleave job.
                        if prev is not None:
                            emit_avA(prev[0], prev[1], prev[2])
                        if ki + 3 <= 15:
                            v_tile(ki + 3)
                        if ki in (8, 12):
                            hf = (ki - 8) // 4
                            qkv_chunk(qk2, 2, hf)
                            cols = slice(hf * 1024, (hf + 1) * 1024)
                            nc.gpsimd.dma_start(k2[:, cols], qk2[64:128, cols])
                            nc.gpsimd.dma_start(q2d[64:128, cols], qk2[0:64, cols])
                    ets_half = []
                    for hi, s in ((0, s0), (1, s1)):
                        et = e_p.tile([128, 1024], bf16, tag="e")
                        acc = st_p.tile([128, 1], f32, tag="acc")
                        nc.scalar.activation(
                            et[:, 0:hl], s[:, 0:hl], EXP,
                            scale=SCALE, accum_out=acc[:],
                        )
                        ets_half.append(et)
                        accs[hi].append(acc)
                    cur_ets.append((h0, hl, ets_half[0], ets_half[1]))
                vpts = [make_vpt(accs[hi], ki, hi) for hi in range(2)]
                # flush completed 512-col blocks (block f done after AV(4f+3),
                # which was emitted during ki=4f+4's hv0 above)
                if causal and ki >= 4 and (ki % 4) == 0:
                    flushA(ki // 4 - 1, avA)
                prev = (ki, cur_ets, vpts)
            emit_avA(prev[0], prev[1], prev[2])
            if causal:
                flushA(3, avA)
            else:
                for f in range(4):
                    flushA(f, avA)
            psA_ctx.__exit__(None, None, None)

            # ---- wave B: head 2, ki pairs in row groups, col-half-packed AV
            psB_ctx = tc.tile_pool(name="psB", bufs=1, space="PSUM")
            psB = psB_ctx.__enter__()
            psO_ctx = tc.tile_pool(name="psO", bufs=2, space="PSUM")
            psO = psO_ctx.__enter__()
            av2 = psB.tile([128, 1024], f32, tag="av2")

            def emit_av2(ki, ets, vpt):
                # col-half packing: abs cols [0,1024) -> av2[0:64],
                # [1024,2048) -> av2[64:128, col-1024]; chunks may not
                # cross the 1024 boundary.
                for (h0, hl, et) in ets:
                    off = 0
                    while off < hl:
                        col = h0 + off
                        n = min(512, hl - off)
                        if col < 1024:
                            n = min(n, 1024 - col)
                            dst = av2[0:64, col : col + n]
                        else:
                            dst = av2[64:128, col - 1024 : col - 1024 + n]
                        nc.tensor.matmul(
                            dst,
                            vpt[:],
                            et[:, off : off + n],
                            start=(ki == 0),
                            stop=(ki == 15),
                            skip_group_check=True,
                        )
                        off += n

            def oproj(f):
                cols = slice(512 * f, 512 * (f + 1))
                src_t = agA_t + agB_t
                for mc, (m0, mw) in enumerate([(0, 128), (128, 64)]):
                    pt = psO.tile([128, 512], f32, tag="po")
                    for dt_i in range(6):
                        nc.tensor.matmul(
                            pt[0:mw, :],
                            wout_t[dt_i][:, m0 : m0 + mw],
                            src_t[dt_i][:, cols],
                            start=(dt_i == 0), stop=(dt_i == 5),
                        )
                    oT = o_p.tile([128, 512], f32, tag="oT")
                    nc.vector.tensor_scalar_add(
                        oT[0:mw, :], pt[0:mw, :], bout_c[0:mw, mc : mc + 1]
                    )
                    nc.gpsimd.dma_start(out[m0 : m0 + mw, cols], oT[0:mw, :])

            srcs = {}
            for ki in range(16):
                if ki % 2 == 0:
                    srcs[ki] = (k2, 0, qk2, 0)
                else:
                    srcs[ki] = (qk2, 64, q2d, 64)

            prevB = None  # (kis, {ki: (ets, vpt)})
            for t in range(8):
                kis = (2 * t, 2 * t + 1)
                ets = {ki: [] for ki in kis}
                accs = {ki: [] for ki in kis}
                maxhv = max(len(halves_of(ki)) for ki in kis)
                for hv in range(maxhv):
                    batch = []
                    for ki in kis:
                        hsk = halves_of(ki)
                        if hv < len(hsk):
                            batch.append((ki, hsk[hv]))
                    s_list = [
                        psS.tile([128, 1024], f32, tag="strip", name="sB")
                        for _ in batch
                    ]
                    maxhl = max(hl for _, (_, hl) in batch)
                    off = 0
                    while off < maxhl:
                        for s, (ki, (h0, hl)) in zip(s_list, batch):
                            if off >= hl:
                                continue
                            n = min(512, hl - off)
                            kT, kb, qT, qb = srcs[ki]
                            diag = causal and hv == 0 and off == 0
                            nc.tensor.matmul(
                                s[:, off : off + n],
                                kT[kb : kb + 64, ki * 128 : (ki + 1) * 128],
                                qT[qb : qb + 64, h0 + off : h0 + off + n],
                                start=True,
                                stop=not diag,
                                skip_group_check=True,
                            )
                        off += 512
                    if causal and hv == 0:
                        for s, (ki, (h0, hl)) in zip(s_list, batch):
                            nc.tensor.matmul(
                                s[:, 0:128], id_b[:], tri_b[:],
                                start=False, stop=True, skip_group_check=True,
                            )
                    if hv == 0:
                        # PE filler: AV of previous pair while exps run
                        if prevB is not None:
                            for ki_p in prevB[0]:
                                e_p_, vpt_p = prevB[1][ki_p]
                                emit_av2(ki_p, e_p_, vpt_p)
                    for s, (ki, (h0, hl)) in zip(s_list, batch):
                        et = e_p.tile([128, 1024], bf16, tag="e")
                        acc = st_p.tile([128, 1], f32, tag="acc")
                        nc.scalar.activation(
                            et[:, 0:hl], s[:, 0:hl], EXP,
                            scale=SCALE, accum_out=acc[:],
                        )
                        ets[ki].append((h0, hl, et))
                        accs[ki].append(acc)
                cur = {}
                for ki in kis:
                    vpt = make_vpt(accs[ki], ki, 2)
                    cur[ki] = (ets[ki], vpt)
                if causal and t >= 2 and t % 2 == 0:
                    flushB(t // 2 - 1, av2)
                prevB = (kis, cur)
            for ki_p in prevB[0]:
                e_p_, vpt_p = prevB[1][ki_p]
                emit_av2(ki_p, e_p_, vpt_p)
            if causal:
                flushB(3, av2)
                for f in range(4):
                    oproj(f)
            else:
                for f in range(4):
                    flushB(f, av2)
                for f in range(4):
                    oproj(f)
            psO_ctx.__exit__(None, None, None)
            psB_ctx.__exit__(None, None, None)
    nc.compile()
    return nc


def _shards(x, mask, W_in, b_in, W_out, b_out):
    """Build per-core input maps (host-side sharding / layout prep)."""
    tri_np = np.where(
        np.arange(128)[None, :] < np.arange(128)[:, None], np.float32(NEG), 0.0
    ).astype(np.float32)
    # split-AllGather row order: rank pairs (h=3r,3r+1) then solos (h=3r+2)
    head_order = [0, 1, 3, 4, 6, 7, 9, 10, 2, 5, 8, 11]
    row_perm = np.concatenate([np.arange(h * 64, (h + 1) * 64) for h in head_order])
    in_maps = []
    for c in range(NCORES):
        b = c // GROUPS
        g = c % GROUPS
        hs = [3 * g, 3 * g + 1, 3 * g + 2]
        qc = [W_in[:, 64 * h : 64 * (h + 1)] for h in hs]
        kc = [W_in[:, D + 64 * h : D + 64 * (h + 1)] for h in hs]
        vc = W_in[:, 2 * D + 64 * hs[0] : 2 * D + 64 * (hs[2] + 1)]
        qb = [b_in[64 * h : 64 * (h + 1)] for h in hs]
        kb = [b_in[D + 64 * h : D + 64 * (h + 1)] for h in hs]
        vb = b_in[2 * D + 64 * hs[0] : 2 * D + 64 * (hs[2] + 1)]
        wqk = np.concatenate(
            [kc[0], kc[1], qc[0], qc[1], qc[2], kc[2]], axis=1
        ).astype(np.float32)
        bqk = np.concatenate([kb[0], kb[1], qb[0], qb[1], qb[2], kb[2]])
        in_maps.append(
            {
                "xT": np.ascontiguousarray(x[b].T, dtype=np.float32),
                "wqk": np.ascontiguousarray(wqk),
                "wv": np.ascontiguousarray(vc, dtype=np.float32),
                "bqkc": np.ascontiguousarray(bqk[:, None], dtype=np.float32),
                "bv": np.ascontiguousarray(vb[None, :], dtype=np.float32),
                "wout": np.ascontiguousarray(
                    W_out[row_perm, EPG * g : EPG * (g + 1)], dtype=np.float32
                ),
                "boutc": np.ascontiguousarray(
                    b_out[EPG * g : EPG * (g + 1), None], dtype=np.float32
                ),
                "tri": tri_np,
                "ident": np.eye(128, dtype=np.float32),
            }
        )
    return in_maps


def _numpy_ref(x, mask, W_in, b_in, W_out, b_out):
    qkv = x @ W_in + b_in
    q, k, v = np.split(qkv, 3, axis=2)
    q = q.reshape(B, S, H, DH).transpose(0, 2, 1, 3)
    k = k.reshape(B, S, H, DH).transpose(0, 2, 1, 3)
    v = v.reshape(B, S, H, DH).transpose(0, 2, 1, 3)
    attn = np.einsum("bhqd,bhkd->bhqk", q, k) / np.sqrt(np.float32(D))
    attn = np.where(mask == 0, -np.inf, attn)
    attn = attn - attn.max(axis=-2, keepdims=True)
    e = np.exp(attn)
    attn = e / e.sum(axis=-2, keepdims=True)
    out = np.einsum("bhqk,bhkd->bhqd", attn, v)
    out = out.transpose(0, 2, 1, 3).reshape(B, S, D)
    return (out @ W_out + b_out).astype(np.float32)


def _run(inputs, trace=False):
    from concourse.bass_utils import run_bass_kernel_spmd

    x = np.asarray(inputs["x"], dtype=np.float32)
    mask = np.asarray(inputs["mask"])
    W_in = np.asarray(inputs["W_in"], dtype=np.float32)
    b_in = np.asarray(inputs["b_in"], dtype=np.float32)
    W_out = np.asarray(inputs["W_out"], dtype=np.float32)
    b_out = np.asarray(inputs["b_out"], dtype=np.float32)

    m2 = np.asarray(mask).reshape(S, S)
    if np.array_equal(m2, np.tril(np.ones((S, S), m2.dtype))):
        causal = True
    elif np.array_equal(m2, np.ones((S, S), m2.dtype)):
        causal = False
    else:
        return _numpy_ref(x, mask, W_in, b_in, W_out, b_out), None

    key = ("nc", causal)
    if key not in _cache:
        _cache[key] = _build(causal)
    nc = _cache[key]

    in_maps = _shards(x, mask, W_in, b_in, W_out, b_out)
    res = run_bass_kernel_spmd(nc, in_maps, core_ids=list(range(NCORES)), trace=trace)

    full = np.empty((B, S, D), dtype=np.float32)
    for c in range(NCORES):
        b, g = c // GROUPS, c % GROUPS
        full[b, :, EPG * g : EPG * (g + 1)] = res.results[c]["out"].T
    return full, res


def kernel(**inputs) -> np.ndarray:
    out, _ = _run(inputs, trace=False)
    return out
